# revision 26
# baseline (speedup 1.0000x reference)
"""Trainium2 Bass kernel for nn_Net_53360673685530 (dehazing SGD loop).

Layout: columns -> partitions (128 groups of 8 cols), rows -> free dim,
c-major per partition so column planes are contiguous runs for the
SBUF->SBUF halo DMAs.  Each core holds a [128, 8, 136] fp32 window: 128
owned rows + 4 halo rows top/bottom; halo errors decay before reaching
owned rows, so NO exchanges are needed for 100 iterations.  Stencil
neighbor columns/rows live in halo slots of extended tiles (LE2 has
2-wide column halos so the whole DX->U->GX chain is local once one DMA
lands; Sc carries 1-wide halos DMA'd with a full iteration of slack),
so every stencil is ONE full-tile op and the T-update cycle crosses only
one DMA.  The sig recompute is lagged by TWO iterations (sig(T_{k-2})
drives update k; 9.5e-4 rel-err cost), which gives the sig chain two
periods of slack.  The 6 raster-wrap fix pixels are dropped (7e-5).
"""
import sys

for _p in ("/opt/trn_rl_repo", "/root/.axon_site/_ro/trn_rl_repo"):
    if _p not in sys.path:
        sys.path.insert(0, _p)

import numpy as np

import concourse.bass as bass
import concourse.tile as tile
from concourse import bacc, mybir, bass_utils, dve_ops
from concourse.dve_spec import Spec, Src0, Src1, sq, lower, _has_src1
from concourse.dve_spec import C0 as DC0, C1 as DC1
from concourse.dve_uop import DveOpSpec
from concourse.dve_ops import DveOp

FP = mybir.dt.float32
AF = mybir.ActivationFunctionType
ALU = mybir.AluOpType

# ---------------------------------------------------------------------------
# Steer the act-table placement pass: the kernel only uses Ln and Exp, and
# exactly one table set ("natural_log_exp_and_others") holds both.  The rust
# placement pass greedily picks the first set containing each function, which
# makes every Ln<->Exp switch reload tables (1.3us each).  Understate every
# other set's contents so the pass must pick the combined set for both
# functions; set indices are preserved, so act_func_set_id stays valid.
import concourse.hw_specs as _hw_specs
import concourse.bacc as _bacc_mod

_COMBINED_SET = "natural_log_exp_and_others"
_orig_get_tables = _hw_specs.get_activation_tables


def _patched_get_tables(arch):
    tabs = _orig_get_tables(arch)
    out = {}
    for name, s in tabs.items():
        if name == _COMBINED_SET:
            out[name] = set(s)
        else:
            out[name] = {f for f in s if f not in (AF.Ln, AF.Exp)}
    return out


_bacc_mod.get_activation_tables = _patched_get_tables

# ---------------------------------------------------------------------------
HP = WP = 1017
PATCH = 7
RATE = 0.001
C2R = 2.0 * RATE
N_ITERS = 100
NCORES = 8
H = 4                 # halo rows each side
OWNR = 128            # owned rows per core
R = OWNR + 2 * H      # window rows = 136
R2 = R + 2            # row-extended (1 zero row each side)
C = 8                 # cols per partition; 128*8 = 1024 >= 1017
NPAD = 1024
LN48 = float(np.log(48.0))
MBBIG = np.float32(1.0e38)

_NC_CACHE = {}
LAST_RESULTS = None


def _register_dve_op(name, spec):
    if name in dve_ops._SUB_OPCODE_FOR_NAME:
        return next(o for o in dve_ops.OPS if o.name == name)
    row = dve_ops._CUSTOM_DVE_ROW_BASE + len(dve_ops.OPS)
    assert row < 0x20
    shas = {}
    for ver in ("v3", "v4"):
        try:
            s = DveOpSpec(name=name, opcode=row, uops=lower(spec, ver=ver),
                          rd1_en=_has_src1(spec))
            shas[ver] = s.sha(ver)
        except Exception:
            pass
    op = DveOp(name, spec, subdim=False, uops_sha=shas)
    dve_ops.OPS.append(op)
    dve_ops._SUB_OPCODE_FOR_NAME[name] = row
    dve_ops.CUSTOM_DVE_SPECS[name] = spec
    return op


# out = ((in0 - in1) * s0 + s1)^2   (s0/s1: literal or [P,1] AP)
SQD = _register_dve_op("SQD_ANT", Spec(
    body=sq((Src0 - Src1) * DC0 + DC1),
    reference=lambda in0, in1, s0, s1, imm2:
        ((in0.astype(np.float32) - in1) * s0 + s1) ** 2,
))


# --------------------------- host-side helpers -----------------------------
def _window_sig(Twin, N0w, N1w, N2w, A, valid):
    """sig for a [R, WP] window (matches kernel math, fixes dropped; fp32)."""
    A = A.astype(np.float32)
    c01 = np.float32(0.5) * (A[1] - A[2])
    c21 = np.float32(0.5) * (A[2] - A[0])
    c20 = np.float32(0.5) * (A[0] - A[1])
    RT = (1.0 / Twin).astype(np.float32)
    P0, P1, P2 = N0w * RT, N1w * RT, N2w * RT
    P2m = np.empty_like(P2)
    P2m[:, 1:] = P2[:, :-1]
    P2m[1:, 0] = P2[:-1, -1]
    P2m[0, 0] = 0.0
    P0p = np.empty_like(P0)
    P0p[:, :-1] = P0[:, 1:]
    P0p[:-1, -1] = P0[1:, 0]
    P0p[-1, -1] = 0.0
    X0 = (np.float32(0.5) * (P1 - P2m) + c01) ** 2
    X1 = (np.float32(0.5) * (P2 - P0) + c21) ** 2
    X2 = (np.float32(0.5) * (P0p - P1) + c20) ** 2
    SS = X0 + X1 + X2
    l2 = np.sqrt(SS)
    with np.errstate(over="ignore"):
        sig = 1.0 / (1.0 + np.exp(np.float32(48.0) * (l2 - np.float32(0.1))))
    return (sig * valid).astype(np.float32)


def _pack(a2d, pad_val):
    """[R, WP] -> [128, C, R] (cols j = 8p + c; c-major per partition)."""
    full = np.full((R, NPAD), pad_val, np.float32)
    full[:, :WP] = a2d
    return np.ascontiguousarray(full.reshape(R, 128, C).transpose(1, 2, 0))


def _pack_ext(a2d):
    """[R, WP] -> [128, C+2, R] with 1-col halos (zeros beyond the edges)."""
    full = np.zeros((R, NPAD + 2), np.float32)
    full[:, 1:WP + 1] = a2d
    ext = np.empty((128, C + 2, R), np.float32)
    for p in range(128):
        ext[p] = full[:, 8 * p:8 * p + 10].T
    return np.ascontiguousarray(ext)


def _core_inputs(core, img, A):
    r0 = OWNR * core - H
    g = r0 + np.arange(R)
    valid = ((g >= 0) & (g < HP)).astype(np.float32)[:, None]
    rows = np.clip(g, 0, HP - 1)
    center = img[PATCH // 2:PATCH // 2 + HP, PATCH // 2:PATCH // 2 + WP, :]
    tlb = np.max(1.0 - center / A, axis=-1).astype(np.float32)

    N0w = (img[rows, :WP, 0] - A[0]) * valid
    N1w = (img[rows, :WP, 1] - A[1]) * valid
    N2w = (img[rows, :WP, 2] - A[2]) * valid
    Tw = np.where(valid > 0, tlb[rows], 1.0).astype(np.float32)
    sc0 = _window_sig(Tw, N0w, N1w, N2w, A, valid)

    mb = np.where(valid > 0, np.float32(1.0), MBBIG)
    mb2 = np.broadcast_to(mb, (R, WP))
    mc2r = np.broadcast_to(np.float32(C2R) * valid, (R, WP))

    return {
        "t0_in": _pack(Tw, 1.0),
        "sc0_in": _pack_ext(sc0),
        "n0_in": _pack(N0w, 0.0),
        "n1_in": _pack(N1w, 0.0),
        "n2_in": _pack(N2w, 0.0),
        "mb_in": _pack(mb2, 1.0),
        "mc2r_in": _pack(mc2r, 0.0),
    }


# ------------------------------ kernel build -------------------------------
def _build(n_iters, A):
    A = np.asarray(A, np.float32)
    key = (n_iters, A.tobytes())
    if key in _NC_CACHE:
        return _NC_CACHE[key]

    c01 = float(np.float32(0.5) * (A[1] - A[2]))
    c21 = float(np.float32(0.5) * (A[2] - A[0]))
    c20 = float(np.float32(0.5) * (A[0] - A[1]))

    nc = bacc.Bacc("TRN2", target_bir_lowering=False, debug=False,
                   num_devices=NCORES)
    for _cv in (LN48, -4.8):
        _ck = (FP, float(_cv))
        if _ck not in nc.const_aps.aps:
            _t = nc.alloc_sbuf_tensor(f"const-f32-{_cv}", [128, 1], FP)
            nc.gpsimd.memset(_t.ap(), float(_cv))
            nc.const_aps.aps[_ck] = _t.ap()
    nc.all_engine_barrier()
    t0_in = nc.dram_tensor("t0_in", [128, C, R], FP, kind="ExternalInput")
    sc0_in = nc.dram_tensor("sc0_in", [128, C + 2, R], FP,
                            kind="ExternalInput")
    n0_in = nc.dram_tensor("n0_in", [128, C, R], FP, kind="ExternalInput")
    n1_in = nc.dram_tensor("n1_in", [128, C, R], FP, kind="ExternalInput")
    n2_in = nc.dram_tensor("n2_in", [128, C, R], FP, kind="ExternalInput")
    mb_in = nc.dram_tensor("mb_in", [128, C, R], FP, kind="ExternalInput")
    mc2r_in = nc.dram_tensor("mc2r_in", [128, C, R], FP, kind="ExternalInput")
    out_dram = nc.dram_tensor("out", [3, 128, C, OWNR], FP,
                              kind="ExternalOutput")

    with tile.TileContext(nc) as tc:
        with (
            tc.tile_pool(name="stat", bufs=1) as stat,
            tc.tile_pool(name="state", bufs=2) as state,
            tc.tile_pool(name="scst", bufs=3) as scst,
            tc.tile_pool(name="rtp", bufs=3) as rtp,
            tc.tile_pool(name="work", bufs=2) as work,
        ):
            N0 = stat.tile([128, C, R], FP)
            N1 = stat.tile([128, C, R], FP)
            N2 = stat.tile([128, C, R], FP)
            MB = stat.tile([128, C, R], FP)
            MC2R = stat.tile([128, C, R], FP)
            nc.sync.dma_start(N0[:], n0_in[:])
            nc.sync.dma_start(N1[:], n1_in[:])
            nc.sync.dma_start(N2[:], n2_in[:])
            nc.sync.dma_start(MB[:], mb_in[:])
            nc.sync.dma_start(MC2R[:], mc2r_in[:])

            T = state.tile([128, C, R], FP, tag="T")
            nc.sync.dma_start(T[:], t0_in[:])

            # pre-zero rotating buffers of halo-extended tiles so
            # never-written halo cells read as the reference's zero padding
            EXT = (("LE2", [128, C + 4, R2]), ("UE", [128, C + 2, R]),
                   ("VE", [128, C, R2]), ("P2E", [128, C + 1, R]),
                   ("P0E", [128, C + 1, R]))
            for _rep in range(2):
                for tg, shp in EXT:
                    t_ = work.tile(shp, FP, tag=tg)
                    nc.gpsimd.memset(t_[:], 0.0)
            sc_q = []
            for _rep in range(3):
                t_ = scst.tile([128, C + 2, R], FP, tag="Sc")
                nc.gpsimd.memset(t_[:], 0.0)
                sc_q.append(t_)
            # lag-2: A(0) and A(1) both use sig(T0)
            nc.sync.dma_start(sc_q[0][:], sc0_in[:])
            nc.sync.dma_start(sc_q[1][:], sc0_in[:])
            sc_q = sc_q[:2]

            LDAT = slice(2, 10)    # data cols inside LE2
            DAT = slice(1, 9)      # data cols inside UE / ScE
            RD = slice(1, R + 1)   # data rows inside LE2 / VE

            for it in range(n_iters):
                # ---------- L and column-halo DMA ----------
                LE2 = work.tile([128, C + 4, R2], FP, tag="LE2")
                nc.scalar.activation(LE2[:, LDAT, RD], T[:], AF.Ln)
                RT1 = rtp.tile([128, C, R], FP, tag="RT1")
                nc.scalar.activation(RT1[:], LE2[:, LDAT, RD], AF.Exp,
                                     scale=-1.0)
                RTcm = work.tile([128, C, R], FP, tag="chE")
                nc.gpsimd.tensor_tensor(RTcm[:], RT1[:], MC2R[:], ALU.mult)
                nc.scalar.dma_start(LE2[1:128, 0:2, RD], LE2[0:127, 8:10, RD])
                nc.scalar.dma_start(LE2[0:127, 10:12, RD], LE2[1:128, 2:4, RD])

                ScE = sc_q.pop(0)   # sig(T_{it-2})

                if it < n_iters - 2:
                    P1 = work.tile([128, C, R], FP, tag="P1")
                    nc.vector.tensor_tensor(P1[:], N1[:], RT1[:], ALU.mult)
                    P0E = work.tile([128, C + 1, R], FP, tag="P0E")
                    nc.gpsimd.tensor_tensor(P0E[:, 0:8, :], N0[:], RT1[:],
                                            ALU.mult)
                    P2E = work.tile([128, C + 1, R], FP, tag="P2E")
                    nc.gpsimd.tensor_tensor(P2E[:, 1:9, :], N2[:], RT1[:],
                                            ALU.mult)
                    nc.sync.dma_start(P2E[1:128, 0:1, :], P2E[0:127, 8:9, :])
                    nc.sync.dma_start(P2E[0:1, 0:1, 1:R],
                                      P2E[127:128, 1:2, 0:R - 1])
                    nc.sync.dma_start(P0E[0:127, 8:9, :], P0E[1:128, 0:1, :])
                    # raster wrap for col 1016 (partition 127, c-out 0): its
                    # X2 input slot is P0E[127, 1] (a pad column) — fill it
                    # with P0[row r+1, col 0] from partition 0.
                    nc.sync.dma_start(P0E[127:128, 1:2, 0:R - 1],
                                      P0E[0:1, 0:1, 1:R])

                # ---------- A phase: T update with lag-2 Sc ----------
                DXE = work.tile([128, C + 2, R], FP, tag="DXE")
                nc.vector.tensor_tensor(DXE[:], LE2[:, 2:12, RD],
                                        LE2[:, 0:10, RD], ALU.subtract)
                UE = work.tile([128, C + 2, R], FP, tag="UE")
                nc.vector.tensor_tensor(UE[:], DXE[:], ScE[:], ALU.mult)
                GX = work.tile([128, C, R], FP, tag="chA")
                nc.vector.tensor_tensor(GX[:], UE[:, 0:8, :], UE[:, 2:10, :],
                                        ALU.subtract)

                DY = work.tile([128, C, R], FP, tag="chB")
                nc.gpsimd.tensor_tensor(DY[:], LE2[:, LDAT, 0:R2 - 2],
                                        LE2[:, LDAT, 2:R2], ALU.subtract)
                VE = work.tile([128, C, R2], FP, tag="VE")
                nc.gpsimd.tensor_tensor(VE[:, :, RD], DY[:], ScE[:, DAT, :],
                                        ALU.mult)
                GY = work.tile([128, C, R], FP, tag="chB")
                nc.gpsimd.tensor_tensor(GY[:], VE[:, :, 2:R2],
                                        VE[:, :, 0:R2 - 2], ALU.subtract)
                GS = work.tile([128, C, R], FP, tag="chB")
                nc.gpsimd.tensor_tensor(GS[:], GX[:], GY[:], ALU.add)
                G2 = work.tile([128, C, R], FP, tag="chB")
                nc.gpsimd.tensor_tensor(G2[:], GS[:], RTcm[:], ALU.mult)
                Tn = state.tile([128, C, R], FP, tag="T")
                nc.gpsimd.tensor_tensor(Tn[:], T[:], G2[:], ALU.subtract)

                # ---------- B: X's, SS, sig chain ----------
                if it < n_iters - 2:
                    X1 = work.tile([128, C, R], FP, tag="chC")
                    nc.vector._custom_dve(SQD, out=X1[:], in0=P2E[:, 1:9, :],
                                          in1=P0E[:, 0:8, :], s0=0.5, s1=c21)
                    X0 = work.tile([128, C, R], FP, tag="chC")
                    nc.vector._custom_dve(SQD, out=X0[:], in0=P1[:],
                                          in1=P2E[:, 0:8, :], s0=0.5, s1=c01)
                    X2 = work.tile([128, C, R], FP, tag="chC")
                    nc.vector._custom_dve(SQD, out=X2[:], in0=P0E[:, 1:9, :],
                                          in1=P1[:], s0=0.5, s1=c20)
                    SSa = work.tile([128, C, R], FP, tag="chD")
                    nc.vector.tensor_tensor(SSa[:], X0[:], X1[:], ALU.add)
                    SS = work.tile([128, C, R], FP, tag="chC")
                    nc.gpsimd.tensor_tensor(SS[:], SSa[:], X2[:], ALU.add)

                    LSS = work.tile([128, C, R], FP, tag="chD")
                    nc.scalar.activation(LSS[:], SS[:], AF.Ln)
                    R48 = work.tile([128, C, R], FP, tag="chD")
                    nc.scalar.activation(R48[:], LSS[:], AF.Exp, bias=LN48,
                                         scale=0.5)
                    E = work.tile([128, C, R], FP, tag="chD")
                    nc.scalar.activation(E[:], R48[:], AF.Exp, bias=-4.8,
                                         scale=1.0)
                    A1p = work.tile([128, C, R], FP, tag="chD")
                    nc.gpsimd.tensor_tensor(A1p[:], E[:], MB[:], ALU.add)
                    ScN = scst.tile([128, C + 2, R], FP, tag="Sc")
                    nc.vector.reciprocal(ScN[:, DAT, :], A1p[:])
                    nc.sync.dma_start(ScN[1:128, 0:1, :], ScN[0:127, 8:9, :])
                    nc.sync.dma_start(ScN[0:127, 9:10, :], ScN[1:128, 1:2, :])
                    sc_q.append(ScN)

                T = Tn

            # ---------------- final output: N/T + A --------------------
            RO = slice(H, H + OWNR)
            RTf = work.tile([128, C, R], FP, tag="chB")
            nc.vector.reciprocal(RTf[:, :, RO], T[:, :, RO])
            for ch, (Nt, Ac) in enumerate([(N0, float(A[0])),
                                           (N1, float(A[1])),
                                           (N2, float(A[2]))]):
                O = work.tile([128, C, R], FP, tag="chA")
                nc.gpsimd.tensor_tensor(O[:, :, RO], Nt[:, :, RO],
                                        RTf[:, :, RO], ALU.mult)
                nc.vector.tensor_scalar(O[:, :, RO], O[:, :, RO], Ac, None,
                                        ALU.add)
                nc.sync.dma_start(out_dram[ch, :, :, :], O[:, :, RO])

    nc.compile()
    _NC_CACHE[key] = nc
    return nc


# ------------------------------- entry point -------------------------------
def kernel(img, airlight, patch_size):
    global LAST_RESULTS
    img = np.ascontiguousarray(np.asarray(img, dtype=np.float32))
    A = np.asarray(airlight, dtype=np.float32)
    p = int(patch_size)
    assert p == PATCH and img.shape == (1024, 1024, 3)

    in_maps = [_core_inputs(c, img, A) for c in range(NCORES)]
    nc = _build(N_ITERS, A)
    res = bass_utils.run_bass_kernel_spmd(nc, in_maps,
                                          core_ids=list(range(NCORES)))
    LAST_RESULTS = res

    out = np.empty((HP, WP, 3), np.float32)
    for c in range(NCORES):
        o = res.results[c]["out"]          # [3, 128, C, OWNR]
        nrows = min(OWNR, HP - OWNR * c)
        for ch in range(3):
            x = o[ch].transpose(2, 0, 1).reshape(OWNR, NPAD)[:, :WP]
            out[OWNR * c:OWNR * c + nrows, :, ch] = x[:nrows]
    return out


if __name__ == "__main__":
    d = np.load("/root/problem/ref_data.npz")
    out = kernel(d["img"], d["airlight"], 7)
    ref = np.load("/root/problem/ref_out.npy")
    err = np.abs(out - ref)
    print("max abs", err.max(), "l2rel",
          np.linalg.norm(out - ref) / np.linalg.norm(ref))


# revision 27
# speedup vs baseline: 1.2392x; 1.2392x over previous
"""Trainium2 Bass kernel for nn_Net_53360673685530 (dehazing SGD loop).

Layout: columns -> partitions (128 groups of 8 cols), rows -> free dim,
c-major per partition so column planes are contiguous runs for the
SBUF->SBUF halo DMAs.  Each core holds a [128, 8, 136] fp32 window: 128
owned rows + 4 halo rows top/bottom; halo errors decay before reaching
owned rows, so NO exchanges are needed for 100 iterations.  Stencil
neighbor columns/rows live in halo slots of extended tiles (LE2 has
2-wide column halos so the whole DX->U->GX chain is local once one DMA
lands; Sc carries 1-wide halos DMA'd with a full iteration of slack),
so every stencil is ONE full-tile op and the T-update cycle crosses only
one DMA.  The sig recompute is lagged by TWO iterations (sig(T_{k-2})
drives update k; 9.5e-4 rel-err cost), which gives the sig chain two
periods of slack.  The 6 raster-wrap fix pixels are dropped (7e-5).
"""
import sys

for _p in ("/opt/trn_rl_repo", "/root/.axon_site/_ro/trn_rl_repo"):
    if _p not in sys.path:
        sys.path.insert(0, _p)

import numpy as np

import concourse.bass as bass
import concourse.tile as tile
from concourse import bacc, mybir, bass_utils, dve_ops
from concourse.dve_spec import Spec, Src0, Src1, sq, lower, _has_src1
from concourse.dve_spec import C0 as DC0, C1 as DC1
from concourse.dve_uop import DveOpSpec
from concourse.dve_ops import DveOp

FP = mybir.dt.float32
AF = mybir.ActivationFunctionType
ALU = mybir.AluOpType

# ---------------------------------------------------------------------------
# Steer the act-table placement pass: the kernel only uses Ln and Exp, and
# exactly one table set ("natural_log_exp_and_others") holds both.  The rust
# placement pass greedily picks the first set containing each function, which
# makes every Ln<->Exp switch reload tables (1.3us each).  Understate every
# other set's contents so the pass must pick the combined set for both
# functions; set indices are preserved, so act_func_set_id stays valid.
import concourse.hw_specs as _hw_specs
import concourse.bacc as _bacc_mod

_COMBINED_SET = "natural_log_exp_and_others"
_orig_get_tables = _hw_specs.get_activation_tables


def _patched_get_tables(arch):
    tabs = _orig_get_tables(arch)
    out = {}
    for name, s in tabs.items():
        if name == _COMBINED_SET:
            out[name] = set(s)
        else:
            out[name] = {f for f in s if f not in (AF.Ln, AF.Exp)}
    return out


_bacc_mod.get_activation_tables = _patched_get_tables

# ---------------------------------------------------------------------------
HP = WP = 1017
PATCH = 7
RATE = 0.001
C2R = 2.0 * RATE
N_ITERS = 100
NCORES = 8
H = 4                 # halo rows each side
OWNR = 128            # owned rows per core
R = OWNR + 2 * H      # window rows = 136
R2 = R + 2            # row-extended (1 zero row each side)
C = 8                 # cols per partition; 128*8 = 1024 >= 1017
NPAD = 1024
LN48 = float(np.log(48.0))
MBBIG = np.float32(1.0e38)

_NC_CACHE = {}
LAST_RESULTS = None


def _register_dve_op(name, spec):
    if name in dve_ops._SUB_OPCODE_FOR_NAME:
        return next(o for o in dve_ops.OPS if o.name == name)
    row = dve_ops._CUSTOM_DVE_ROW_BASE + len(dve_ops.OPS)
    assert row < 0x20
    shas = {}
    for ver in ("v3", "v4"):
        try:
            s = DveOpSpec(name=name, opcode=row, uops=lower(spec, ver=ver),
                          rd1_en=_has_src1(spec))
            shas[ver] = s.sha(ver)
        except Exception:
            pass
    op = DveOp(name, spec, subdim=False, uops_sha=shas)
    dve_ops.OPS.append(op)
    dve_ops._SUB_OPCODE_FOR_NAME[name] = row
    dve_ops.CUSTOM_DVE_SPECS[name] = spec
    return op


# out = ((in0 - in1) * s0 + s1)^2   (s0/s1: literal or [P,1] AP)
SQD = _register_dve_op("SQD_ANT", Spec(
    body=sq((Src0 - Src1) * DC0 + DC1),
    reference=lambda in0, in1, s0, s1, imm2:
        ((in0.astype(np.float32) - in1) * s0 + s1) ** 2,
))


# --------------------------- host-side helpers -----------------------------
def _window_sig(Twin, N0w, N1w, N2w, A, valid):
    """sig for a [R, WP] window (matches kernel math, fixes dropped; fp32)."""
    A = A.astype(np.float32)
    c01 = np.float32(0.5) * (A[1] - A[2])
    c21 = np.float32(0.5) * (A[2] - A[0])
    c20 = np.float32(0.5) * (A[0] - A[1])
    RT = (1.0 / Twin).astype(np.float32)
    P0, P1, P2 = N0w * RT, N1w * RT, N2w * RT
    P2m = np.empty_like(P2)
    P2m[:, 1:] = P2[:, :-1]
    P2m[1:, 0] = P2[:-1, -1]
    P2m[0, 0] = 0.0
    P0p = np.empty_like(P0)
    P0p[:, :-1] = P0[:, 1:]
    P0p[:-1, -1] = P0[1:, 0]
    P0p[-1, -1] = 0.0
    X0 = (np.float32(0.5) * (P1 - P2m) + c01) ** 2
    X1 = (np.float32(0.5) * (P2 - P0) + c21) ** 2
    X2 = (np.float32(0.5) * (P0p - P1) + c20) ** 2
    SS = X0 + X1 + X2
    l2 = np.sqrt(SS)
    with np.errstate(over="ignore"):
        sig = 1.0 / (1.0 + np.exp(np.float32(48.0) * (l2 - np.float32(0.1))))
    return (sig * valid).astype(np.float32)


def _pack(a2d, pad_val):
    """[R, WP] -> [128, C, R] (cols j = 8p + c; c-major per partition)."""
    full = np.full((R, NPAD), pad_val, np.float32)
    full[:, :WP] = a2d
    return np.ascontiguousarray(full.reshape(R, 128, C).transpose(1, 2, 0))


def _pack_ext(a2d):
    """[R, WP] -> [128, C+2, R] with 1-col halos (zeros beyond the edges)."""
    full = np.zeros((R, NPAD + 2), np.float32)
    full[:, 1:WP + 1] = a2d
    ext = np.empty((128, C + 2, R), np.float32)
    for p in range(128):
        ext[p] = full[:, 8 * p:8 * p + 10].T
    return np.ascontiguousarray(ext)


def _core_inputs(core, img, A):
    r0 = OWNR * core - H
    g = r0 + np.arange(R)
    valid = ((g >= 0) & (g < HP)).astype(np.float32)[:, None]
    rows = np.clip(g, 0, HP - 1)
    center = img[PATCH // 2:PATCH // 2 + HP, PATCH // 2:PATCH // 2 + WP, :]
    tlb = np.max(1.0 - center / A, axis=-1).astype(np.float32)

    N0w = (img[rows, :WP, 0] - A[0]) * valid
    N1w = (img[rows, :WP, 1] - A[1]) * valid
    N2w = (img[rows, :WP, 2] - A[2]) * valid
    Tw = np.where(valid > 0, tlb[rows], 1.0).astype(np.float32)
    sc0 = _window_sig(Tw, N0w, N1w, N2w, A, valid)

    mb = np.where(valid > 0, np.float32(1.0), MBBIG)
    mb2 = np.broadcast_to(mb, (R, WP))
    mc2r = np.broadcast_to(np.float32(C2R) * valid, (R, WP))

    return {
        "t0_in": _pack(Tw, 1.0),
        "sc0_in": _pack_ext(sc0),
        "n0_in": _pack(N0w, 0.0),
        "n1_in": _pack(N1w, 0.0),
        "n2_in": _pack(N2w, 0.0),
        "mb_in": _pack(mb2, 1.0),
        "mc2r_in": _pack(mc2r, 0.0),
    }


# ------------------------------ kernel build -------------------------------
def _build(n_iters, A):
    A = np.asarray(A, np.float32)
    key = (n_iters, A.tobytes())
    if key in _NC_CACHE:
        return _NC_CACHE[key]

    c01 = float(np.float32(0.5) * (A[1] - A[2]))
    c21 = float(np.float32(0.5) * (A[2] - A[0]))
    c20 = float(np.float32(0.5) * (A[0] - A[1]))

    nc = bacc.Bacc("TRN2", target_bir_lowering=False, debug=False,
                   num_devices=NCORES)
    for _cv in (LN48, -4.8):
        _ck = (FP, float(_cv))
        if _ck not in nc.const_aps.aps:
            _t = nc.alloc_sbuf_tensor(f"const-f32-{_cv}", [128, 1], FP)
            nc.gpsimd.memset(_t.ap(), float(_cv))
            nc.const_aps.aps[_ck] = _t.ap()
    nc.all_engine_barrier()
    t0_in = nc.dram_tensor("t0_in", [128, C, R], FP, kind="ExternalInput")
    sc0_in = nc.dram_tensor("sc0_in", [128, C + 2, R], FP,
                            kind="ExternalInput")
    n0_in = nc.dram_tensor("n0_in", [128, C, R], FP, kind="ExternalInput")
    n1_in = nc.dram_tensor("n1_in", [128, C, R], FP, kind="ExternalInput")
    n2_in = nc.dram_tensor("n2_in", [128, C, R], FP, kind="ExternalInput")
    mb_in = nc.dram_tensor("mb_in", [128, C, R], FP, kind="ExternalInput")
    mc2r_in = nc.dram_tensor("mc2r_in", [128, C, R], FP, kind="ExternalInput")
    out_dram = nc.dram_tensor("out", [3, 128, C, OWNR], FP,
                              kind="ExternalOutput")

    with tile.TileContext(nc) as tc:
        with (
            tc.tile_pool(name="stat", bufs=1) as stat,
            tc.tile_pool(name="state", bufs=2) as state,
            tc.tile_pool(name="scst", bufs=3) as scst,
            tc.tile_pool(name="rtp", bufs=3) as rtp,
            tc.tile_pool(name="work", bufs=2) as work,
        ):
            N0 = stat.tile([128, C, R], FP)
            N1 = stat.tile([128, C, R], FP)
            N2 = stat.tile([128, C, R], FP)
            MB = stat.tile([128, C, R], FP)
            MC2R = stat.tile([128, C, R], FP)
            nc.sync.dma_start(N0[:], n0_in[:])
            nc.sync.dma_start(N1[:], n1_in[:])
            nc.sync.dma_start(N2[:], n2_in[:])
            nc.sync.dma_start(MB[:], mb_in[:])
            nc.sync.dma_start(MC2R[:], mc2r_in[:])

            T = state.tile([128, C, R], FP, tag="T")
            nc.sync.dma_start(T[:], t0_in[:])

            # pre-zero rotating buffers of halo-extended tiles so
            # never-written halo cells read as the reference's zero padding
            EXT = (("LE2", [128, C + 4, R2]), ("UE", [128, C + 2, R]),
                   ("VE", [128, C, R2]), ("P2E", [128, C + 1, R]),
                   ("P0E", [128, C + 1, R]))
            for _rep in range(2):
                for tg, shp in EXT:
                    t_ = work.tile(shp, FP, tag=tg)
                    nc.gpsimd.memset(t_[:], 0.0)
            sc_q = []
            for _rep in range(3):
                t_ = scst.tile([128, C + 2, R], FP, tag="Sc")
                nc.gpsimd.memset(t_[:], 0.0)
                sc_q.append(t_)
            # lag-2: A(0) and A(1) both use sig(T0)
            nc.sync.dma_start(sc_q[0][:], sc0_in[:])
            nc.sync.dma_start(sc_q[1][:], sc0_in[:])
            sc_q = sc_q[:2]

            LDAT = slice(2, 10)    # data cols inside LE2
            DAT = slice(1, 9)      # data cols inside UE / ScE
            RD = slice(1, R + 1)   # data rows inside LE2 / VE

            for it in range(n_iters):
                # ---------- L and column-halo DMA ----------
                LE2 = work.tile([128, C + 4, R2], FP, tag="LE2")
                nc.scalar.activation(LE2[:, LDAT, RD], T[:], AF.Ln)
                RT1 = rtp.tile([128, C, R], FP, tag="RT1")
                nc.scalar.activation(RT1[:], LE2[:, LDAT, RD], AF.Exp,
                                     scale=-1.0)
                RTcm = work.tile([128, C, R], FP, tag="chE")
                nc.gpsimd.tensor_tensor(RTcm[:], RT1[:], MC2R[:], ALU.mult)
                nc.sync.dma_start(LE2[1:128, 0:2, RD], LE2[0:127, 8:10, RD])
                nc.sync.dma_start(LE2[0:127, 10:12, RD], LE2[1:128, 2:4, RD])

                ScE = sc_q.pop(0)   # sig(T_{it-2})

                if it < n_iters - 2:
                    P1 = work.tile([128, C, R], FP, tag="P1")
                    nc.vector.tensor_tensor(P1[:], N1[:], RT1[:], ALU.mult)
                    P0E = work.tile([128, C + 1, R], FP, tag="P0E")
                    nc.gpsimd.tensor_tensor(P0E[:, 0:8, :], N0[:], RT1[:],
                                            ALU.mult)
                    P2E = work.tile([128, C + 1, R], FP, tag="P2E")
                    nc.gpsimd.tensor_tensor(P2E[:, 1:9, :], N2[:], RT1[:],
                                            ALU.mult)
                    nc.sync.dma_start(P2E[1:128, 0:1, :], P2E[0:127, 8:9, :])
                    nc.sync.dma_start(P2E[0:1, 0:1, 1:R],
                                      P2E[127:128, 1:2, 0:R - 1])
                    nc.sync.dma_start(P0E[0:127, 8:9, :], P0E[1:128, 0:1, :])
                    # raster wrap for col 1016 (partition 127, c-out 0): its
                    # X2 input slot is P0E[127, 1] (a pad column) — fill it
                    # with P0[row r+1, col 0] from partition 0.
                    nc.sync.dma_start(P0E[127:128, 1:2, 0:R - 1],
                                      P0E[0:1, 0:1, 1:R])

                # ---------- A phase: T update with lag-2 Sc ----------
                DXE = work.tile([128, C + 2, R], FP, tag="DXE")
                nc.vector.tensor_tensor(DXE[:], LE2[:, 2:12, RD],
                                        LE2[:, 0:10, RD], ALU.subtract)
                UE = work.tile([128, C + 2, R], FP, tag="UE")
                nc.vector.tensor_tensor(UE[:], DXE[:], ScE[:], ALU.mult)
                GX = work.tile([128, C, R], FP, tag="chA")
                nc.vector.tensor_tensor(GX[:], UE[:, 0:8, :], UE[:, 2:10, :],
                                        ALU.subtract)

                DY = work.tile([128, C, R], FP, tag="chB")
                nc.gpsimd.tensor_tensor(DY[:], LE2[:, LDAT, 0:R2 - 2],
                                        LE2[:, LDAT, 2:R2], ALU.subtract)
                VE = work.tile([128, C, R2], FP, tag="VE")
                nc.gpsimd.tensor_tensor(VE[:, :, RD], DY[:], ScE[:, DAT, :],
                                        ALU.mult)
                GY = work.tile([128, C, R], FP, tag="chB")
                nc.gpsimd.tensor_tensor(GY[:], VE[:, :, 2:R2],
                                        VE[:, :, 0:R2 - 2], ALU.subtract)
                GS = work.tile([128, C, R], FP, tag="chB")
                nc.gpsimd.tensor_tensor(GS[:], GX[:], GY[:], ALU.add)
                G2 = work.tile([128, C, R], FP, tag="chB")
                nc.gpsimd.tensor_tensor(G2[:], GS[:], RTcm[:], ALU.mult)
                Tn = state.tile([128, C, R], FP, tag="T")
                nc.gpsimd.tensor_tensor(Tn[:], T[:], G2[:], ALU.subtract)

                # ---------- B: X's, SS, sig chain ----------
                if it < n_iters - 2:
                    X1 = work.tile([128, C, R], FP, tag="chC")
                    nc.vector._custom_dve(SQD, out=X1[:], in0=P2E[:, 1:9, :],
                                          in1=P0E[:, 0:8, :], s0=0.5, s1=c21)
                    X0 = work.tile([128, C, R], FP, tag="chC")
                    nc.vector._custom_dve(SQD, out=X0[:], in0=P1[:],
                                          in1=P2E[:, 0:8, :], s0=0.5, s1=c01)
                    X2 = work.tile([128, C, R], FP, tag="chC")
                    nc.vector._custom_dve(SQD, out=X2[:], in0=P0E[:, 1:9, :],
                                          in1=P1[:], s0=0.5, s1=c20)
                    SSa = work.tile([128, C, R], FP, tag="chD")
                    nc.vector.tensor_tensor(SSa[:], X0[:], X1[:], ALU.add)
                    SS = work.tile([128, C, R], FP, tag="chC")
                    nc.gpsimd.tensor_tensor(SS[:], SSa[:], X2[:], ALU.add)

                    LSS = work.tile([128, C, R], FP, tag="chD")
                    nc.scalar.activation(LSS[:], SS[:], AF.Ln)
                    R48 = work.tile([128, C, R], FP, tag="chD")
                    nc.scalar.activation(R48[:], LSS[:], AF.Exp, bias=LN48,
                                         scale=0.5)
                    E = work.tile([128, C, R], FP, tag="chD")
                    nc.scalar.activation(E[:], R48[:], AF.Exp, bias=-4.8,
                                         scale=1.0)
                    A1p = work.tile([128, C, R], FP, tag="chD")
                    nc.gpsimd.tensor_tensor(A1p[:], E[:], MB[:], ALU.add)
                    ScN = scst.tile([128, C + 2, R], FP, tag="Sc")
                    nc.vector.reciprocal(ScN[:, DAT, :], A1p[:])
                    nc.sync.dma_start(ScN[1:128, 0:1, :], ScN[0:127, 8:9, :])
                    nc.sync.dma_start(ScN[0:127, 9:10, :], ScN[1:128, 1:2, :])
                    sc_q.append(ScN)

                T = Tn

            # ---------------- final output: N/T + A --------------------
            RO = slice(H, H + OWNR)
            RTf = work.tile([128, C, R], FP, tag="chB")
            nc.vector.reciprocal(RTf[:, :, RO], T[:, :, RO])
            for ch, (Nt, Ac) in enumerate([(N0, float(A[0])),
                                           (N1, float(A[1])),
                                           (N2, float(A[2]))]):
                O = work.tile([128, C, R], FP, tag="chA")
                nc.gpsimd.tensor_tensor(O[:, :, RO], Nt[:, :, RO],
                                        RTf[:, :, RO], ALU.mult)
                nc.vector.tensor_scalar(O[:, :, RO], O[:, :, RO], Ac, None,
                                        ALU.add)
                nc.sync.dma_start(out_dram[ch, :, :, :], O[:, :, RO])

    nc.compile()
    _NC_CACHE[key] = nc
    return nc


# ------------------------------- entry point -------------------------------
def kernel(img, airlight, patch_size):
    global LAST_RESULTS
    img = np.ascontiguousarray(np.asarray(img, dtype=np.float32))
    A = np.asarray(airlight, dtype=np.float32)
    p = int(patch_size)
    assert p == PATCH and img.shape == (1024, 1024, 3)

    in_maps = [_core_inputs(c, img, A) for c in range(NCORES)]
    nc = _build(N_ITERS, A)
    res = bass_utils.run_bass_kernel_spmd(nc, in_maps,
                                          core_ids=list(range(NCORES)))
    LAST_RESULTS = res

    out = np.empty((HP, WP, 3), np.float32)
    for c in range(NCORES):
        o = res.results[c]["out"]          # [3, 128, C, OWNR]
        nrows = min(OWNR, HP - OWNR * c)
        for ch in range(3):
            x = o[ch].transpose(2, 0, 1).reshape(OWNR, NPAD)[:, :WP]
            out[OWNR * c:OWNR * c + nrows, :, ch] = x[:nrows]
    return out


if __name__ == "__main__":
    d = np.load("/root/problem/ref_data.npz")
    out = kernel(d["img"], d["airlight"], 7)
    ref = np.load("/root/problem/ref_out.npy")
    err = np.abs(out - ref)
    print("max abs", err.max(), "l2rel",
          np.linalg.norm(out - ref) / np.linalg.norm(ref))
